# revision 2
# baseline (speedup 1.0000x reference)
"""Trainium2 Bass kernel for topk_masking row-parallel linear.

Reference semantics:
    idx  = argmax_k(score[o, i, :])            (first index wins ties)
    net  = weight[o, i, idx]                   [OUT, IN]
    out  = x @ net.T                           [BATCH, OUT]

Device algorithm (per core, o-shard of 256 out-features):
    layout [i, (k, o)]  k-OUTER (host pre-transposed; i on partitions)
    m   = max_k(s)            fp32 TT max-tree (3 stages, DVE) — exact argmax
    t   = bf16(s - m)         GPSIMD pass (<= 0, == 0 only at argmax; bf16
                              keeps fp32 exponent range so tiny gaps stay < 0)
    v   = t + w'              bf16 TT add, DVE 2x mode (w' = w * 2^-34 bf16:
                              at argmax t==0 -> v = w' exactly; elsewhere
                              v ~ t << w'_argmax)
    net'= max_k(v)            bf16 TT max-tree (3 stages, DVE) == w'_sel
    outT[o, b] += net'.T @ xt on the PE in bf16 (x pre-scaled by 2^34 on the
                              host cancels the 2^-34), fp32 PSUM accumulate

Selection is exact unless the top-2 score gap < 2*std*2^-34 (prob ~1e-9 per
slot); bf16 weight/x quantization gives ~2.4e-3 scale-relative output error
(validated against the fp32 reference in numpy).
"""

import sys

import numpy as np

if "/opt/trn_rl_repo" not in sys.path:
    sys.path.insert(0, "/opt/trn_rl_repo")

import ml_dtypes

import concourse.bacc as bacc
import concourse.tile as tile
from concourse import mybir
from concourse.bass_utils import run_bass_kernel_spmd

OUT_F, IN_F, K, BATCH = 2048, 2048, 8, 256
N_CORES = 8
OSH = OUT_F // N_CORES  # 256 out-features per core
P = 128
NBLK = IN_F // P        # 16 contraction blocks
FREE = OSH * K          # 2048 elements per partition row of an s/w block
F32 = mybir.dt.float32
BF16 = mybir.dt.bfloat16
ALU = mybir.AluOpType
BF16_NP = ml_dtypes.bfloat16

# o-columns [0, SUB_DVE) of the subtract pass run on the DVE, the rest on
# GPSIMD.
SUB_DVE = 0


def build(sub_dve=SUB_DVE, io_bufs=3, mid_bufs=3):
    nc = bacc.Bacc("TRN2", target_bir_lowering=False, debug=False)
    s_d = nc.dram_tensor("s", [IN_F, FREE], F32, kind="ExternalInput")
    w_d = nc.dram_tensor("w", [IN_F, FREE], BF16, kind="ExternalInput")
    x_d = nc.dram_tensor("xt", [IN_F, BATCH], BF16, kind="ExternalInput")
    o_d = nc.dram_tensor("outT", [OSH, BATCH], F32, kind="ExternalOutput")

    s_blk = s_d.ap().rearrange("(n p) f -> n p f", p=P)
    w_blk = w_d.ap().rearrange("(n p) f -> n p f", p=P)
    x_blk = x_d.ap().rearrange("(n p) b -> p n b", p=P)
    o_blk = o_d.ap().rearrange("(h p) b -> h p b", p=P)

    H = FREE // 2   # 1024
    Q = FREE // 4   # 512

    with tile.TileContext(nc) as tc:
        with (
            tc.tile_pool(name="io", bufs=io_bufs) as io,
            tc.tile_pool(name="mid", bufs=mid_bufs) as mid,
            tc.tile_pool(name="tree", bufs=mid_bufs) as tr,
            tc.tile_pool(name="stat", bufs=1) as stat,
            tc.tile_pool(name="ps", bufs=1, space="PSUM") as psp,
        ):
            xt_sb = stat.tile([P, NBLK * BATCH], BF16)
            xt3 = xt_sb[:].rearrange("p (n b) -> p n b", b=BATCH)
            nc.scalar.dma_start(xt3, x_blk)

            ps0 = psp.tile([P, BATCH], F32)
            ps1 = psp.tile([P, BATCH], F32)

            for n in range(NBLK):
                s_sb = io.tile([P, FREE], F32)
                w_sb = io.tile([P, FREE], BF16)
                nc.sync.dma_start(s_sb[:], s_blk[n])
                nc.scalar.dma_start(w_sb[:], w_blk[n])

                # fp32 max-tree over k (k-outer: halves are k 0-3 vs 4-7)
                m4 = tr.tile([P, H], F32)
                nc.vector.tensor_tensor(m4[:], s_sb[:, 0:H], s_sb[:, H:FREE], ALU.max)
                m2 = tr.tile([P, Q], F32)
                nc.vector.tensor_tensor(m2[:], m4[:, 0:Q], m4[:, Q:H], ALU.max)
                m = tr.tile([P, OSH], F32)
                nc.vector.tensor_tensor(m[:], m2[:, 0:OSH], m2[:, OSH:Q], ALU.max)

                # t = bf16(s - m), m broadcast over k
                t_sb = mid.tile([P, FREE], BF16)
                t3 = t_sb[:].rearrange("p (k o) -> p k o", k=K)
                s3 = s_sb[:].rearrange("p (k o) -> p k o", k=K)
                mb = m[:].unsqueeze(1).broadcast_to([P, K, OSH])
                c0 = sub_dve
                if c0 > 0:
                    nc.vector.tensor_tensor(
                        t3[:, :, :c0], s3[:, :, :c0], mb[:, :, :c0], ALU.subtract
                    )
                if c0 < OSH:
                    nc.gpsimd.tensor_tensor(
                        t3[:, :, c0:], s3[:, :, c0:], mb[:, :, c0:], ALU.subtract
                    )

                # v = t + w'  (bf16 2x mode, fully contiguous)
                v_sb = mid.tile([P, FREE], BF16)
                nc.vector.tensor_tensor(v_sb[:], t_sb[:], w_sb[:], ALU.add)

                # bf16 max-tree over k -> net' = w'_sel
                u4 = tr.tile([P, H], BF16)
                nc.vector.tensor_tensor(u4[:], v_sb[:, 0:H], v_sb[:, H:FREE], ALU.max)
                u2 = tr.tile([P, Q], BF16)
                nc.vector.tensor_tensor(u2[:], u4[:, 0:Q], u4[:, Q:H], ALU.max)
                net = tr.tile([P, OSH], BF16)
                nc.vector.tensor_tensor(net[:], u2[:, 0:OSH], u2[:, OSH:Q], ALU.max)

                nc.tensor.matmul(
                    ps0[:], net[:, 0:P], xt3[:, n, :],
                    start=(n == 0), stop=(n == NBLK - 1),
                )
                nc.tensor.matmul(
                    ps1[:], net[:, P:OSH], xt3[:, n, :],
                    start=(n == 0), stop=(n == NBLK - 1),
                )

            ob0 = stat.tile([P, BATCH], F32)
            ob1 = stat.tile([P, BATCH], F32)
            nc.scalar.copy(ob0[:], ps0[:])
            nc.scalar.copy(ob1[:], ps1[:])
            nc.sync.dma_start(o_blk[0], ob0[:])
            nc.sync.dma_start(o_blk[1], ob1[:])
    nc.compile()
    return nc


def make_in_maps(x, weight, score):
    # k-outer transposes: [O, I, K] -> [I, K, O]; exact power-of-2 pre-scaling
    # w' = w * 2^-34 (bf16), x' = x * 2^34 (bf16) cancels in the matmul.
    w_t = np.transpose(
        np.asarray(weight, dtype=np.float32) * np.float32(2.0**-34), (1, 2, 0)
    ).astype(BF16_NP)                                                # [IN, K, OUT]
    s_t = np.transpose(np.asarray(score, dtype=np.float32), (1, 2, 0))
    xt = np.ascontiguousarray(
        (np.asarray(x, dtype=np.float32) * np.float32(2.0**34)).T
    ).astype(BF16_NP)                                                # [IN, BATCH]
    in_maps = []
    for c in range(N_CORES):
        sl = slice(c * OSH, (c + 1) * OSH)
        in_maps.append(
            {
                "w": np.ascontiguousarray(w_t[:, :, sl]).reshape(IN_F, FREE),
                "s": np.ascontiguousarray(s_t[:, :, sl]).reshape(IN_F, FREE),
                "xt": xt,
            }
        )
    return in_maps


def assemble_out(results):
    outT = np.concatenate([results[c]["outT"] for c in range(N_CORES)], axis=0)
    return np.ascontiguousarray(outT.T)  # [BATCH, OUT]


def run(x, weight, score, trace=False, nc=None):
    """Returns (out, BassKernelResults)."""
    if nc is None:
        nc = build()
    res = run_bass_kernel_spmd(
        nc, make_in_maps(x, weight, score), list(range(N_CORES)), trace=trace
    )
    return assemble_out(res.results), res


def kernel(x, weight, score):
    out, _ = run(x, weight, score, trace=False)
    return out


# revision 4
# speedup vs baseline: 1.3568x; 1.3568x over previous
"""Trainium2 Bass kernel for topk_masking row-parallel linear.

Reference semantics:
    idx  = argmax_k(score[o, i, :])            (first index wins ties)
    net  = weight[o, i, idx]                   [OUT, IN]
    out  = x @ net.T                           [BATCH, OUT]

Device algorithm (per core, o-shard of 256 out-features):
    layout [i, (k, o)]  k-OUTER (host pre-transposed; i on partitions)
    m   = max_k(s)            fp32 TT max-tree (3 stages) — exact argmax
    t   = bf16(s - m)         fp32 TT, bf16 out (<= 0, == 0 only at argmax;
                              bf16 keeps the fp32 exponent range so tiny
                              gaps stay strictly negative)
    v   = t + w'              bf16 TT add, DVE 2x mode (w' = w * 2^-34 bf16:
                              at argmax t==0 -> v = w' exactly; elsewhere
                              v ~ t << w'_argmax)
    net'= max_k(v)            bf16 TT max-tree (2x mode) == w'_sel
    outT[o, b] += net'.T @ xt on the PE in bf16 (x pre-scaled by 2^34 on the
                              host cancels the 2^-34), fp32 PSUM accumulate

All selection passes run on the DVE: GPSIMD shares the DVE SBUF port, so
concurrent Q7 streaming serializes against 2-port DVE ops (measured: zero
net gain).  CHUNK amortizes the ~151-cycle per-instruction overhead.

Selection is exact unless the top-2 score gap < 2*std*2^-34 (prob ~1e-9 per
slot); bf16 weight/x quantization gives ~2.4e-3 scale-relative output error
(validated against the fp32 reference in numpy).
"""

import sys

import numpy as np

if "/opt/trn_rl_repo" not in sys.path:
    sys.path.insert(0, "/opt/trn_rl_repo")

import ml_dtypes

import concourse.bacc as bacc
import concourse.tile as tile
from concourse import mybir
from concourse.bass_utils import run_bass_kernel_spmd

OUT_F, IN_F, K, BATCH = 2048, 2048, 8, 256
N_CORES = 8
OSH = OUT_F // N_CORES  # 256 out-features per core
P = 128
NBLK = IN_F // P        # 16 contraction blocks
FREE = OSH * K          # 2048 elements per partition row of an s/w block
F32 = mybir.dt.float32
BF16 = mybir.dt.bfloat16
ALU = mybir.AluOpType
BF16_NP = ml_dtypes.bfloat16

CHUNK = 2


def build(chunk=CHUNK, io_bufs=3, mid_bufs=2):
    nc = bacc.Bacc("TRN2", target_bir_lowering=False, debug=False)
    s_d = nc.dram_tensor("s", [IN_F, FREE], F32, kind="ExternalInput")
    w_d = nc.dram_tensor("w", [IN_F, FREE], BF16, kind="ExternalInput")
    x_d = nc.dram_tensor("xt", [IN_F, BATCH], BF16, kind="ExternalInput")
    o_d = nc.dram_tensor("outT", [OSH, BATCH], F32, kind="ExternalOutput")

    s_blk = s_d.ap().rearrange("(n c p) f -> n p c f", p=P, c=chunk)
    w_blk = w_d.ap().rearrange("(n c p) f -> n p c f", p=P, c=chunk)
    x_blk = x_d.ap().rearrange("(n p) b -> p n b", p=P)
    o_blk = o_d.ap().rearrange("(h p) b -> h p b", p=P)

    CF = chunk * FREE
    H, Q = FREE // 2, FREE // 4   # 1024, 512 within one block

    with tile.TileContext(nc) as tc:
        with (
            tc.tile_pool(name="io", bufs=io_bufs) as io,
            tc.tile_pool(name="mid", bufs=mid_bufs) as mid,
            tc.tile_pool(name="tree", bufs=mid_bufs) as tr,
            tc.tile_pool(name="stat", bufs=1) as stat,
            tc.tile_pool(name="ps", bufs=1, space="PSUM") as psp,
        ):
            xt_sb = stat.tile([P, NBLK * BATCH], BF16)
            xt3 = xt_sb[:].rearrange("p (n b) -> p n b", b=BATCH)
            nc.scalar.dma_start(xt3, x_blk)

            ps0 = psp.tile([P, BATCH], F32)
            ps1 = psp.tile([P, BATCH], F32)

            for n in range(NBLK // chunk):
                s_sb = io.tile([P, CF], F32)
                w_sb = io.tile([P, CF], BF16)
                nc.sync.dma_start(
                    s_sb[:].rearrange("p (c f) -> p c f", c=chunk), s_blk[n]
                )
                nc.scalar.dma_start(
                    w_sb[:].rearrange("p (c f) -> p c f", c=chunk), w_blk[n]
                )

                # 3D views: [P, chunk, elems-in-block]
                s3 = s_sb[:].rearrange("p (c f) -> p c f", c=chunk)

                # fp32 max-tree over k (k-outer: halves are k 0-3 vs 4-7)
                m4 = tr.tile([P, chunk * H], F32)
                m4c = m4[:].rearrange("p (c f) -> p c f", c=chunk)
                nc.vector.tensor_tensor(
                    m4c, s3[:, :, 0:H], s3[:, :, H:FREE], ALU.max
                )
                m2 = tr.tile([P, chunk * Q], F32)
                m2c = m2[:].rearrange("p (c f) -> p c f", c=chunk)
                nc.vector.tensor_tensor(
                    m2c, m4c[:, :, 0:Q], m4c[:, :, Q:H], ALU.max
                )
                m = tr.tile([P, chunk * OSH], F32)
                mc = m[:].rearrange("p (c f) -> p c f", c=chunk)
                nc.vector.tensor_tensor(
                    mc, m2c[:, :, 0:OSH], m2c[:, :, OSH:Q], ALU.max
                )

                # t = bf16(s - m), m broadcast over k
                t_sb = mid.tile([P, CF], BF16)
                t4 = t_sb[:].rearrange("p (c k o) -> p c k o", c=chunk, k=K)
                s4 = s3.rearrange("p c (k o) -> p c k o", k=K)
                mb = mc.unsqueeze(2).broadcast_to([P, chunk, K, OSH])
                nc.vector.tensor_tensor(t4, s4, mb, ALU.subtract)

                # v = t + w'  (bf16 2x mode, fully contiguous)
                v_sb = mid.tile([P, CF], BF16)
                nc.vector.tensor_tensor(v_sb[:], t_sb[:], w_sb[:], ALU.add)

                # bf16 max-tree over k -> net' = w'_sel
                v3 = v_sb[:].rearrange("p (c f) -> p c f", c=chunk)
                u4 = tr.tile([P, chunk * H], BF16)
                u4c = u4[:].rearrange("p (c f) -> p c f", c=chunk)
                nc.vector.tensor_tensor(
                    u4c, v3[:, :, 0:H], v3[:, :, H:FREE], ALU.max
                )
                u2 = tr.tile([P, chunk * Q], BF16)
                u2c = u2[:].rearrange("p (c f) -> p c f", c=chunk)
                nc.vector.tensor_tensor(
                    u2c, u4c[:, :, 0:Q], u4c[:, :, Q:H], ALU.max
                )
                net = tr.tile([P, chunk * OSH], BF16)
                netc = net[:].rearrange("p (c f) -> p c f", c=chunk)
                nc.vector.tensor_tensor(
                    netc, u2c[:, :, 0:OSH], u2c[:, :, OSH:Q], ALU.max
                )

                for cc in range(chunk):
                    blk = n * chunk + cc
                    nc.tensor.matmul(
                        ps0[:], netc[:, cc, 0:P], xt3[:, blk, :],
                        start=(blk == 0), stop=(blk == NBLK - 1),
                    )
                    nc.tensor.matmul(
                        ps1[:], netc[:, cc, P:OSH], xt3[:, blk, :],
                        start=(blk == 0), stop=(blk == NBLK - 1),
                    )

            ob0 = stat.tile([P, BATCH], F32)
            ob1 = stat.tile([P, BATCH], F32)
            nc.scalar.copy(ob0[:], ps0[:])
            nc.scalar.copy(ob1[:], ps1[:])
            nc.sync.dma_start(o_blk[0], ob0[:])
            nc.sync.dma_start(o_blk[1], ob1[:])
    nc.compile()
    return nc


def make_in_maps(x, weight, score):
    # k-outer transposes: [O, I, K] -> [I, K, O]; exact power-of-2 pre-scaling
    # w' = w * 2^-34 (bf16), x' = x * 2^34 (bf16) cancels in the matmul.
    w_t = np.transpose(
        np.asarray(weight, dtype=np.float32) * np.float32(2.0**-34), (1, 2, 0)
    ).astype(BF16_NP)                                                # [IN, K, OUT]
    s_t = np.transpose(np.asarray(score, dtype=np.float32), (1, 2, 0))
    xt = np.ascontiguousarray(
        (np.asarray(x, dtype=np.float32) * np.float32(2.0**34)).T
    ).astype(BF16_NP)                                                # [IN, BATCH]
    in_maps = []
    for c in range(N_CORES):
        sl = slice(c * OSH, (c + 1) * OSH)
        in_maps.append(
            {
                "w": np.ascontiguousarray(w_t[:, :, sl]).reshape(IN_F, FREE),
                "s": np.ascontiguousarray(s_t[:, :, sl]).reshape(IN_F, FREE),
                "xt": xt,
            }
        )
    return in_maps


def assemble_out(results):
    outT = np.concatenate([results[c]["outT"] for c in range(N_CORES)], axis=0)
    return np.ascontiguousarray(outT.T)  # [BATCH, OUT]


def run(x, weight, score, trace=False, nc=None):
    """Returns (out, BassKernelResults)."""
    if nc is None:
        nc = build()
    res = run_bass_kernel_spmd(
        nc, make_in_maps(x, weight, score), list(range(N_CORES)), trace=trace
    )
    return assemble_out(res.results), res


def kernel(x, weight, score):
    out, _ = run(x, weight, score, trace=False)
    return out


# revision 12
# speedup vs baseline: 1.5130x; 1.1151x over previous
"""Trainium2 Bass kernel for topk_masking row-parallel linear.

Reference semantics:
    idx  = argmax_k(score[o, i, :])            (first index wins ties)
    net  = weight[o, i, idx]                   [OUT, IN]
    out  = x @ net.T                           [BATCH, OUT]

Device algorithm (per core, o-shard of 256 out-features):
    layout [i, (k, o)]  k-OUTER (host pre-transposed; i on partitions)
    m   = max_k(s)            fp32 TT max-tree (3 stages) — exact argmax
    t   = bf16(s - m)         fp32 TT, bf16 out (<= 0, == 0 only at argmax;
                              bf16 keeps the fp32 exponent range so tiny
                              gaps stay strictly negative)
    v   = t + w'              bf16 TT add, DVE 2x mode (w' = w * 2^-34 bf16:
                              at argmax t==0 -> v = w' exactly; elsewhere
                              v ~ t << w'_argmax)
    net'= max_k(v)            bf16 TT max-tree (2x mode) == w'_sel
    outT[o, b] += net'.T @ xt on the PE in bf16 (x pre-scaled by 2^34 on the
                              host cancels the 2^-34), fp32 PSUM accumulate

All selection passes run on the DVE: GPSIMD shares the DVE SBUF port, so
concurrent Q7 streaming serializes against 2-port DVE ops (measured: zero
net gain).  CHUNK amortizes the ~151-cycle per-instruction overhead.

Selection is exact unless the top-2 score gap < 2*std*2^-34 (prob ~1e-9 per
slot); bf16 weight/x quantization gives ~2.4e-3 scale-relative output error
(validated against the fp32 reference in numpy).
"""

import sys

import numpy as np

if "/opt/trn_rl_repo" not in sys.path:
    sys.path.insert(0, "/opt/trn_rl_repo")

import ml_dtypes

import concourse.bacc as bacc
import concourse.tile as tile
from concourse import mybir
from concourse.bass_utils import run_bass_kernel_spmd

OUT_F, IN_F, K, BATCH = 2048, 2048, 8, 256
N_CORES = 8
OSH = OUT_F // N_CORES  # 256 out-features per core
P = 128
NBLK = IN_F // P        # 16 contraction blocks
FREE = OSH * K          # 2048 elements per partition row of an s/w block
F32 = mybir.dt.float32
BF16 = mybir.dt.bfloat16
ALU = mybir.AluOpType
BF16_NP = ml_dtypes.bfloat16

CHUNK = 2


def build(chunk=CHUNK, io_bufs=4, mid_bufs=2):
    nc = bacc.Bacc("TRN2", target_bir_lowering=False, debug=False)
    s_d = nc.dram_tensor("s", [IN_F, FREE], F32, kind="ExternalInput")
    w_d = nc.dram_tensor("w", [IN_F, FREE], BF16, kind="ExternalInput")
    x_d = nc.dram_tensor("xt", [P, NBLK * BATCH], BF16, kind="ExternalInput")
    o_d = nc.dram_tensor("outT", [OSH, BATCH], F32, kind="ExternalOutput")

    s_blk = s_d.ap().rearrange("(n c p) f -> n p c f", p=P, c=chunk)
    w_blk = w_d.ap().rearrange("(n c p) f -> n p c f", p=P, c=chunk)
    # xt is host-preblocked to [P, NBLK*BATCH] so its DMA is contiguous
    x_blk = x_d.ap()
    o_blk = o_d.ap().rearrange("(h p) b -> h p b", p=P)

    CF = chunk * FREE
    H, Q = FREE // 2, FREE // 4   # 1024, 512 within one block

    with tile.TileContext(nc) as tc:
        with (
            tc.tile_pool(name="io", bufs=io_bufs) as io,
            tc.tile_pool(name="mid", bufs=mid_bufs) as mid,
            tc.tile_pool(name="tree", bufs=mid_bufs) as tr,
            tc.tile_pool(name="stat", bufs=1) as stat,
            tc.tile_pool(name="ps", bufs=1, space="PSUM") as psp,
        ):
            xt_sb = stat.tile([P, NBLK * BATCH], BF16)
            xt3 = xt_sb[:].rearrange("p (n b) -> p n b", b=BATCH)
            nc.scalar.dma_start(xt_sb[:], x_blk)

            ps0 = psp.tile([P, BATCH], F32)
            ps1 = psp.tile([P, BATCH], F32)

            for n in range(NBLK // chunk):
                s_sb = io.tile([P, CF], F32)
                w_sb = io.tile([P, CF], BF16)
                nc.sync.dma_start(
                    s_sb[:].rearrange("p (c f) -> p c f", c=chunk), s_blk[n]
                )
                nc.sync.dma_start(
                    w_sb[:].rearrange("p (c f) -> p c f", c=chunk), w_blk[n]
                )

                # 3D views: [P, chunk, elems-in-block]
                s3 = s_sb[:].rearrange("p (c f) -> p c f", c=chunk)

                # fp32 max-tree over k (k-outer: halves are k 0-3 vs 4-7)
                m4 = tr.tile([P, chunk * H], F32)
                m4c = m4[:].rearrange("p (c f) -> p c f", c=chunk)
                nc.vector.tensor_tensor(
                    m4c, s3[:, :, 0:H], s3[:, :, H:FREE], ALU.max
                )
                m2 = tr.tile([P, chunk * Q], F32)
                m2c = m2[:].rearrange("p (c f) -> p c f", c=chunk)
                nc.vector.tensor_tensor(
                    m2c, m4c[:, :, 0:Q], m4c[:, :, Q:H], ALU.max
                )
                m = tr.tile([P, chunk * OSH], F32)
                mc = m[:].rearrange("p (c f) -> p c f", c=chunk)
                nc.vector.tensor_tensor(
                    mc, m2c[:, :, 0:OSH], m2c[:, :, OSH:Q], ALU.max
                )

                # t = bf16(s - m), m broadcast over k
                t_sb = mid.tile([P, CF], BF16)
                t4 = t_sb[:].rearrange("p (c k o) -> p c k o", c=chunk, k=K)
                s4 = s3.rearrange("p c (k o) -> p c k o", k=K)
                mb = mc.unsqueeze(2).broadcast_to([P, chunk, K, OSH])
                nc.vector.tensor_tensor(t4, s4, mb, ALU.subtract)

                # v = t + w'  (bf16 2x mode, fully contiguous)
                v_sb = mid.tile([P, CF], BF16)
                nc.vector.tensor_tensor(v_sb[:], t_sb[:], w_sb[:], ALU.add)

                # bf16 max-tree over k -> net' = w'_sel
                v3 = v_sb[:].rearrange("p (c f) -> p c f", c=chunk)
                u4 = tr.tile([P, chunk * H], BF16)
                u4c = u4[:].rearrange("p (c f) -> p c f", c=chunk)
                nc.vector.tensor_tensor(
                    u4c, v3[:, :, 0:H], v3[:, :, H:FREE], ALU.max
                )
                u2 = tr.tile([P, chunk * Q], BF16)
                u2c = u2[:].rearrange("p (c f) -> p c f", c=chunk)
                nc.vector.tensor_tensor(
                    u2c, u4c[:, :, 0:Q], u4c[:, :, Q:H], ALU.max
                )
                net = tr.tile([P, chunk * OSH], BF16)
                netc = net[:].rearrange("p (c f) -> p c f", c=chunk)
                nc.vector.tensor_tensor(
                    netc, u2c[:, :, 0:OSH], u2c[:, :, OSH:Q], ALU.max
                )

                for cc in range(chunk):
                    blk = n * chunk + cc
                    nc.tensor.matmul(
                        ps0[:], netc[:, cc, 0:P], xt3[:, blk, :],
                        start=(blk == 0), stop=(blk == NBLK - 1),
                    )
                    nc.tensor.matmul(
                        ps1[:], netc[:, cc, P:OSH], xt3[:, blk, :],
                        start=(blk == 0), stop=(blk == NBLK - 1),
                    )

            ob0 = stat.tile([P, BATCH], F32)
            ob1 = stat.tile([P, BATCH], F32)
            nc.scalar.copy(ob0[:], ps0[:])
            nc.scalar.copy(ob1[:], ps1[:])
            nc.sync.dma_start(o_blk[0], ob0[:])
            nc.sync.dma_start(o_blk[1], ob1[:])
    nc.compile()
    return nc


def make_in_maps(x, weight, score):
    # k-outer transposes: [O, I, K] -> [I, K, O]; exact power-of-2 pre-scaling
    # w' = w * 2^-34 (bf16), x' = x * 2^34 (bf16) cancels in the matmul.
    w_t = np.transpose(
        np.asarray(weight, dtype=np.float32) * np.float32(2.0**-34), (1, 2, 0)
    ).astype(BF16_NP)                                                # [IN, K, OUT]
    s_t = np.transpose(np.asarray(score, dtype=np.float32), (1, 2, 0))
    xt = np.ascontiguousarray(
        (np.asarray(x, dtype=np.float32) * np.float32(2.0**34)).T
    ).astype(BF16_NP)                                                # [IN, BATCH]
    # pre-block xt for a contiguous per-partition DMA: [P, NBLK*BATCH] where
    # partition p holds rows {p, P+p, 2P+p, ...} of x.T
    xt = np.ascontiguousarray(
        xt.reshape(NBLK, P, BATCH).transpose(1, 0, 2)
    ).reshape(P, NBLK * BATCH)
    in_maps = []
    for c in range(N_CORES):
        sl = slice(c * OSH, (c + 1) * OSH)
        in_maps.append(
            {
                "w": np.ascontiguousarray(w_t[:, :, sl]).reshape(IN_F, FREE),
                "s": np.ascontiguousarray(s_t[:, :, sl]).reshape(IN_F, FREE),
                "xt": xt,
            }
        )
    return in_maps


def assemble_out(results):
    outT = np.concatenate([results[c]["outT"] for c in range(N_CORES)], axis=0)
    return np.ascontiguousarray(outT.T)  # [BATCH, OUT]


def run(x, weight, score, trace=False, nc=None):
    """Returns (out, BassKernelResults)."""
    if nc is None:
        nc = build()
    res = run_bass_kernel_spmd(
        nc, make_in_maps(x, weight, score), list(range(N_CORES)), trace=trace
    )
    return assemble_out(res.results), res


def kernel(x, weight, score):
    out, _ = run(x, weight, score, trace=False)
    return out
